# revision 1
# baseline (speedup 1.0000x reference)
"""Trainium2 Bass kernel for nn_AdvancedTransformerBlock (B=512, L=60, D=1024, H=16).

Data-parallel over batch across 8 NeuronCores (64 rows/core), no collectives.
Per core, activations are feature-major ([feat, tok]) for all matmuls (bf16 in,
fp32 PSUM); token-major only where per-token scalars are needed (LN apply,
residual, final combine); DMA-xbar transposes bridge the two. LN affines and
the 1/sqrt(hd) scale are folded into weights host-side. Attention: S^T via
K^T-stationary matmuls with PE quadrant concurrency (head parity x row
parity), softmax = exp + ones-matmul row-sum broadcast + reciprocal_approx,
O^T via V-stationary matmuls landing directly feature-major.
"""
import os
import sys

for _p in ("/opt/trn_rl_repo", "/root/.axon_site/_ro/trn_rl_repo"):
    if os.path.isdir(_p) and _p not in sys.path:
        sys.path.insert(0, _p)

from contextlib import ExitStack

import ml_dtypes
import numpy as np

import concourse.bass as bass
import concourse.tile as tile
from concourse import bacc, mybir
from concourse.bass_utils import run_bass_kernel_spmd

BF16 = mybir.dt.bfloat16
F32 = mybir.dt.float32
AF = mybir.ActivationFunctionType
OP = mybir.AluOpType

B, L, D = 512, 60, 1024
NH, HD = 16, 64
NCORES = 8
EPS = 1e-5

ROWS = int(os.environ.get("BASSKERNEL_ROWS", B // NCORES))  # rows per core
SUP_ROWS = 32                      # rows per super-chunk
T_SUP = SUP_ROWS * L               # 1920 tokens per super
NT_SUP = T_SUP // 128              # 15 token tiles per super
T_PAD = 1984                       # xn0T free padding (V-proj M=64 overreach)
G_ROWS = 8                         # rows per attention group
T_G = G_ROWS * L                   # 480 tokens per group
NCH = 480                          # big-matmul free chunk (<=512, 4/super)
FCH = 384                          # F-phase streamed chunk (3 token tiles)


def _bf16(a):
    return np.ascontiguousarray(a.astype(ml_dtypes.bfloat16))


def prep_weights(inp):
    """Fold LN affines + qk scale into weights; pre-transpose to lhsT layout
    [in_feat, out_feat]; cast bf16."""
    f32 = {k: np.asarray(v, np.float64) for k, v in inp.items()}
    w = {}

    def fold_ln(W, ln_w, ln_b, b):
        return W * ln_w[None, :], W @ ln_b + b

    for br in ("short", "long"):
        Wf, bf = fold_ln(f32[f"{br}_wqkv"], f32["ln1_w"], f32["ln1_b"],
                         f32[f"{br}_bqkv"])
        scale = 1.0 / np.sqrt(HD)
        Wf = Wf.copy()
        bf = bf.copy()
        Wf[:D] *= scale
        bf[:D] *= scale
        w[f"wqk_{br}"] = _bf16(Wf[: 2 * D].T)      # [1024, 2048]
        w[f"bqk_{br}"] = bf[: 2 * D].astype(np.float32)
        w[f"wv_{br}"] = _bf16(Wf[2 * D:].T)        # [1024, 1024]
        w[f"bv_{br}"] = bf[2 * D:].astype(np.float32)
        w[f"wo_{br}"] = _bf16(f32[f"{br}_wo"].T)   # [1024, 1024]
        w[f"bo_{br}"] = f32[f"{br}_bo"].astype(np.float32)

    w["wg"] = _bf16(f32["gate_w"].T)               # [2048, 1024]
    w["bg"] = f32["gate_b"].astype(np.float32)
    W1, b1 = fold_ln(f32["ffn_w1"], f32["ln2_w"], f32["ln2_b"], f32["ffn_b1"])
    w["w1"] = _bf16(W1.T)                          # [1024, 2048]
    w["b1"] = b1.astype(np.float32)
    w["w2"] = _bf16(f32["ffn_w2"].T)               # [2048, 1024]
    w["b2"] = f32["ffn_b2"].astype(np.float32)
    for k in ("r", "z", "c"):
        w[f"w{k}"] = _bf16(f32[f"{k}_w"].T)        # [2048, 1024]
        w[f"b{k}"] = f32[f"{k}_b"].astype(np.float32)

    # causal mask [128, 16*64] bf16: partition rslot*64+k, free h*64+q: k<=q
    m = np.zeros((128, NH * 64), np.float32)
    tri = (np.arange(L)[:, None] <= np.arange(L)[None, :]).astype(np.float32)
    for rs in range(2):
        for h in range(NH):
            m[rs * 64: rs * 64 + L, h * 64: h * 64 + L] = tri
    w["maskc"] = _bf16(m)
    return w


def build_nc(w, rows):
    supers = rows // SUP_ROWS
    assert rows % SUP_ROWS == 0
    nc = bacc.Bacc("TRN2", target_bir_lowering=False, debug=False,
                   num_devices=NCORES)
    x_ext = nc.declare_dram_parameter("x", [rows, L, D], F32, isOutput=False)
    out_ext = nc.declare_dram_parameter("out", [rows, L, D], F32, isOutput=True)
    xf = x_ext.ap().flatten_outer_dims()       # [rows*L, 1024]
    of = out_ext.ap().flatten_outer_dims()

    wext = {k: nc.inline_tensor(v, name=k) for k, v in w.items()}
    have_bias = {k: bool(np.any(np.asarray(w[k], np.float32)))
                 for k in w if k.startswith("b")}

    so_spill = nc.dram_tensor("so_spill", [supers, 128, 8, T_SUP], BF16)
    lo_spill = nc.dram_tensor("lo_spill", [supers, 128, 8, T_SUP], BF16)
    z_spill = nc.dram_tensor("z_spill", [supers, 128, 8, T_SUP], BF16)
    x1_spill = nc.dram_tensor("x1_spill", [supers, T_SUP, D], F32)

    with tile.TileContext(nc) as tc, ExitStack() as ctx:
        # ----- global pools -----
        consts = ctx.enter_context(tc.tile_pool(name="consts", bufs=1))
        bpool = ctx.enter_context(tc.tile_pool(name="bpool", bufs=2))
        tokp = ctx.enter_context(tc.tile_pool(name="tokp", bufs=4))
        stat = ctx.enter_context(tc.tile_pool(name="stat", bufs=4))
        mm_ps = ctx.enter_context(tc.tile_pool(name="mmps", bufs=2, space="PSUM"))
        st_ps = ctx.enter_context(tc.tile_pool(name="stps", bufs=1, space="PSUM"))
        rs_ps = ctx.enter_context(tc.tile_pool(name="rsps", bufs=1, space="PSUM"))
        ot_ps = ctx.enter_context(tc.tile_pool(name="otps", bufs=1, space="PSUM"))

        mask_t = consts.tile([128, NH * 64], BF16)
        nc.sync.dma_start(mask_t[:], wext["maskc"].ap())
        ones_t = consts.tile([128, 64], BF16)
        nc.vector.memset(ones_t, 1.0)
        eps_t = consts.tile([128, 1], F32)
        nc.vector.memset(eps_t, EPS)

        def load_w(pool, key, kt, mt, tag="wt"):
            t = pool.tile([128, kt, mt], BF16, tag=tag)
            nc.sync.dma_start(t[:], wext[key].ap().rearrange("(k p) m -> p k m", p=128))
            return t

        def load_bias(key, mt):
            t = bpool.tile([128, mt], F32, tag=f"b{mt}")
            nc.sync.dma_start(t[:], wext[key].ap().rearrange("(m p) -> p m", p=128))
            return t

        def evac(i, out_ap, in_ap, func=AF.Copy, bias=None):
            if func == AF.Copy and bias is None:
                if i % 2 == 0:
                    nc.vector.tensor_copy(out_ap, in_ap)
                else:
                    nc.scalar.copy(out_ap, in_ap)
            elif bias is None:
                nc.scalar.activation(out_ap, in_ap, func)
            else:
                nc.scalar.activation(out_ap, in_ap, func, bias=bias)

        def big_matmul(dst_fn, wt, kts, rhs_fn, ntok, func=AF.Copy,
                       bias_t=None, bias_key=None, mtiles=None):
            """for each m-tile: dst_fn(m) [128, ntok] = act(W[:,m].T @ rhs + b)."""
            if mtiles is None:
                mtiles = wt.shape[2] // 128
            for mi in range(mtiles):
                ps = mm_ps.tile([128, 512], F32, tag="mmps")
                for k in range(kts):
                    nc.tensor.matmul(
                        ps[:, 0:ntok],
                        lhsT=wt[:, k, mi * 128:(mi + 1) * 128],
                        rhs=rhs_fn(k),
                        start=(k == 0), stop=(k == kts - 1))
                b = None
                if bias_key is not None and have_bias.get(bias_key, False):
                    b = bias_t[:, mi: mi + 1]
                evac(mi, dst_fn(mi), ps[:, 0:ntok], func=func, bias=b)

        def ln_stats(xsrc_f32, mvs, tt):
            """bn stats for one token tile -> mvs[:, tt, 0:2] (mean, var)."""
            stt = stat.tile([128, 2, 6], F32, tag="bnst")
            for half in range(2):
                nc.vector.bn_stats(stt[:, half, :],
                                   xsrc_f32[:, half * 512:(half + 1) * 512])
            nc.vector.bn_aggr(mvs[:, tt, :], stt[:])

        def ln_rstd_batch(mvs):
            """mvs[:, :, 1] (var) -> rstd[:, :] for all tiles: 2 ACT ops."""
            rstd = stat.tile([128, NT_SUP], F32, tag="rstdb")
            nc.scalar.activation(rstd[:, :], mvs[:, :, 1], AF.Ln, bias=eps_t[:])
            nc.scalar.activation(rstd[:, :], rstd[:, :], AF.Exp, scale=-0.5)
            return rstd

        def ln_apply(xsrc_f32, dst_bf16, mvs, rstd, tt):
            nc.vector.tensor_scalar(
                out=dst_bf16[:], in0=xsrc_f32[:], scalar1=mvs[:, tt, 0:1],
                scalar2=rstd[:, tt:tt + 1], op0=OP.subtract, op1=OP.mult)

        for s in range(supers):
            sT0 = s * T_SUP
            # ================= Phase A: LN1 + forward transpose =============
            ctxA = ExitStack()
            p_xn0T = ctxA.enter_context(tc.tile_pool(name=f"xn0T{s}", bufs=1))
            xn0T = p_xn0T.tile([128, 8, T_PAD], BF16)
            nc.vector.memset(xn0T[:, :, T_SUP:T_PAD], 0.0)
            mvs = stat.tile([128, NT_SUP, 2], F32, tag="mvsA")
            for tt in range(NT_SUP):
                t0 = sT0 + tt * 128
                xt = tokp.tile([128, D], F32, tag="tokf32")
                nc.sync.dma_start(xt[:], xf[t0:t0 + 128, :])
                ln_stats(xt, mvs, tt)
            rstdA = ln_rstd_batch(mvs)
            for tt in range(NT_SUP):
                t0 = sT0 + tt * 128
                xt = tokp.tile([128, D], F32, tag="tokf32")
                nc.sync.dma_start(xt[:], xf[t0:t0 + 128, :])
                xn = tokp.tile([128, D], BF16, tag="tokbf")
                ln_apply(xt, xn, mvs, rstdA, tt)
                nc.sync.dma_start_transpose(
                    xn0T[:, :, tt * 128:(tt + 1) * 128], xn[:])

            # ================= Phase B: attention branches ==================
            for br in ("short", "long"):
                ctxB = ExitStack()
                wp = ctxB.enter_context(tc.tile_pool(name=f"w{br}{s}", bufs=1))
                gp = ctxB.enter_context(tc.tile_pool(name=f"g{br}{s}", bufs=1))
                p_OT = ctxB.enter_context(tc.tile_pool(name=f"OT{br}{s}", bufs=1))
                attp = ctxB.enter_context(tc.tile_pool(name=f"at{br}{s}", bufs=2))
                bstg = ctxB.enter_context(tc.tile_pool(name=f"bs{br}{s}", bufs=2))
                wqk = load_w(wp, f"wqk_{br}", 8, 2048, tag="wqk")
                wv = load_w(wp, f"wv_{br}", 8, 1024, tag="wvo")
                bqk = load_bias(f"bqk_{br}", 16)
                OT = p_OT.tile([128, 8, T_SUP], BF16)
                for g in range(4):
                    gt0 = g * T_G
                    qkT = gp.tile([128, 16, T_G], BF16, tag="qkT")
                    big_matmul(lambda mi: qkT[:, mi, :], wqk, 8,
                               lambda k: xn0T[:, k, gt0:gt0 + T_G], T_G,
                               bias_t=bqk, bias_key=f"bqk_{br}")
                    Vg = gp.tile([128, 4, 1024], BF16, tag="Vg")
                    for rp in range(4):
                        for c in range(2):
                            vps = mm_ps.tile([128, 512], F32, tag="mmps")
                            for k in range(8):
                                for rsl in range(2):
                                    tok = gt0 + rp * 120 + rsl * 60
                                    nc.tensor.matmul(
                                        vps[rsl * 64: rsl * 64 + 64, :],
                                        lhsT=xn0T[:, k, tok:tok + 64],
                                        rhs=wv[:, k, c * 512:(c + 1) * 512],
                                        start=(k == 0), stop=(k == 7),
                                        skip_group_check=True)
                            nc.vector.tensor_copy(
                                Vg[:, rp, c * 512:(c + 1) * 512], vps[:])
                    for rp in range(4):
                        st = st_ps.tile([128, NH * 64], F32, tag="stps")
                        for rsl in range(2):
                            for h in range(NH):
                                f, par = h // 2, (h % 2) * 64
                                sg = ((h % 2) * 8 + h // 2) * 64
                                tok = rp * 120 + rsl * 60
                                nc.tensor.matmul(
                                    st[rsl * 64: rsl * 64 + L, sg:sg + L],
                                    lhsT=qkT[par:par + HD, 8 + f, tok:tok + L],
                                    rhs=qkT[par:par + HD, f, tok:tok + L],
                                    start=True, stop=True)
                        pt = attp.tile([128, NH, 64], BF16, tag="pt")
                        st3 = st[:, :].rearrange("p (h x) -> p h x", x=64)
                        mt3 = mask_t[:, :].rearrange("p (h x) -> p h x", x=64)
                        for rsl in range(2):
                            sl = slice(rsl * 64, rsl * 64 + L)
                            nc.scalar.activation(pt[sl, :, 0:L], st3[sl, :, 0:L], AF.Exp)
                            nc.vector.memset(pt[sl, :, L:64], 0.0)
                            if br == "short":
                                nc.vector.tensor_mul(pt[sl, :, 0:L], pt[sl, :, 0:L],
                                                     mt3[sl, :, 0:L])
                        rsum = rs_ps.tile([128, NH * 64], F32, tag="rsps")
                        for rsl in range(2):
                            sl = slice(rsl * 64, rsl * 64 + L)
                            for c in range(2):
                                nc.tensor.matmul(
                                    rsum[sl, c * 512:(c + 1) * 512],
                                    lhsT=ones_t[sl, 0:L],
                                    rhs=pt[sl, :, :].rearrange("p h x -> p (h x)")[:, c * 512:(c + 1) * 512],
                                    start=True, stop=True)
                        rcp = attp.tile([128, NH, 64], F32, tag="rcp")
                        ptn = attp.tile([128, NH, 64], BF16, tag="ptn")
                        rs3 = rsum[:, :].rearrange("p (h x) -> p h x", x=64)
                        for rsl in range(2):
                            sl = slice(rsl * 64, rsl * 64 + L)
                            # tracked sliver read orders DVE after the PE row-sum
                            # (the custom-DVE op below is invisible to Tile deps;
                            # DVE itself is in-order, so this fence suffices)
                            nc.vector.tensor_copy(rcp[sl, :, 0:1], rs3[sl, :, 0:1])
                            nc.vector.reciprocal(rcp[sl, :, 0:L], rs3[sl, :, 0:L])
                            nc.vector.tensor_mul(ptn[sl, :, 0:L], pt[sl, :, 0:L], rcp[sl, :, 0:L])
                        ot = ot_ps.tile([128, 8 * 128], F32, tag="otps")
                        for rsl in range(2):
                            for h in range(NH):
                                f, hp = h // 2, h % 2
                                sg = (h % 2) * 8 + h // 2
                                nc.tensor.matmul(
                                    ot[hp * 64: hp * 64 + HD,
                                       rsl * 512 + f * 64: rsl * 512 + f * 64 + L],
                                    lhsT=Vg[rsl * 64: rsl * 64 + L, rp, h * HD:(h + 1) * HD],
                                    rhs=ptn[rsl * 64: rsl * 64 + L, sg, 0:L],
                                    start=True, stop=True)
                        for rsl in range(2):
                            tok0 = gt0 + rp * 120 + rsl * 60
                            nc.scalar.copy(
                                OT[:, :, tok0: tok0 + L],
                                ot[:, rsl * 512:(rsl + 1) * 512].rearrange(
                                    "p (f y) -> p f y", y=64)[:, :, 0:L])
                # out projection
                wo = load_w(wp, f"wo_{br}", 8, 1024, tag="wvo")
                bo = load_bias(f"bo_{br}", 8)
                spill = so_spill if br == "short" else lo_spill
                for c in range(4):
                    ct = slice(c * NCH, (c + 1) * NCH)
                    bo_c = bstg.tile([128, 8, NCH], BF16, tag="bstg")
                    big_matmul(lambda mi: bo_c[:, mi, :], wo, 8,
                               lambda k, ct=ct: OT[:, k, ct], NCH,
                               bias_t=bo, bias_key=f"bo_{br}")
                    nc.sync.dma_start(spill[s, :, :, ct], bo_c[:])
                ctxB.close()
            ctxA.close()   # xn0T dead

            # ================= Phase D: gate + combine + LN2 ================
            p_ffnT = ExitStack()
            pp_ffnT = p_ffnT.enter_context(tc.tile_pool(name=f"ffnT{s}", bufs=1))
            p_xn2T = ExitStack()
            pp_xn2T = p_xn2T.enter_context(tc.tile_pool(name=f"xn2T{s}", bufs=1))
            p_attnT = ExitStack()
            pp_attnT = p_attnT.enter_context(tc.tile_pool(name=f"attnT{s}", bufs=1))
            ctxD = ExitStack()
            wpD = ctxD.enter_context(tc.tile_pool(name=f"wD{s}", bufs=1))
            dstg = ctxD.enter_context(tc.tile_pool(name=f"dstg{s}", bufs=2))
            wg = load_w(wpD, "wg", 16, 1024)
            bg = load_bias("bg", 8)
            attnT = pp_attnT.tile([128, 8, T_SUP], BF16)
            for c in range(4):
                ct = slice(c * NCH, (c + 1) * NCH)
                soc = dstg.tile([128, 8, NCH], BF16, tag="soc")
                nc.sync.dma_start(soc[:], so_spill[s, :, :, ct])
                loc = dstg.tile([128, 8, NCH], BF16, tag="loc")
                nc.sync.dma_start(loc[:], lo_spill[s, :, :, ct])
                gT = dstg.tile([128, 8, NCH], BF16, tag="gT")
                big_matmul(lambda mi: gT[:, mi, :], wg, 16,
                           lambda k: (soc if k < 8 else loc)[:, k % 8, :],
                           NCH, func=AF.Sigmoid, bias_t=bg, bias_key="bg")
                for m in range(8):
                    dm = dstg.tile([128, NCH], BF16, tag="dm")
                    nc.vector.tensor_sub(dm[:], soc[:, m, :], loc[:, m, :])
                    nc.vector.tensor_mul(dm[:], dm[:], gT[:, m, :])
                    nc.vector.tensor_add(attnT[:, m, ct], loc[:, m, :], dm[:])
            ctxD.close()
            xn2T = pp_xn2T.tile([128, 8, T_SUP], BF16)
            mvs2 = stat.tile([128, NT_SUP, 2], F32, tag="mvsD")
            for tt in range(NT_SUP):
                t0 = sT0 + tt * 128
                atok = tokp.tile([128, D], BF16, tag="tokbf")
                for f in range(8):
                    nc.sync.dma_start_transpose(
                        atok[:, f * 128:(f + 1) * 128],
                        attnT[:, f, tt * 128: tt * 128 + 128])
                xt = tokp.tile([128, D], F32, tag="tokf32")
                nc.sync.dma_start(xt[:], xf[t0:t0 + 128, :])
                x1 = tokp.tile([128, D], F32, tag="tokf32")
                nc.vector.tensor_add(x1[:], xt[:], atok[:])
                nc.sync.dma_start(x1_spill[s, tt * 128: tt * 128 + 128, :], x1[:])
                ln_stats(x1, mvs2, tt)
            rstdD = ln_rstd_batch(mvs2)
            for tt in range(NT_SUP):
                x1 = tokp.tile([128, D], F32, tag="tokf32")
                nc.sync.dma_start(x1[:], x1_spill[s, tt * 128: tt * 128 + 128, :])
                xn2 = tokp.tile([128, D], BF16, tag="tokbf")
                ln_apply(x1, xn2, mvs2, rstdD, tt)
                nc.sync.dma_start_transpose(
                    xn2T[:, :, tt * 128:(tt + 1) * 128], xn2[:])
            p_attnT.close()

            # ================= Phase E: FFN =================================
            ctxE = ExitStack()
            wpE = ctxE.enter_context(tc.tile_pool(name=f"wE{s}", bufs=2))
            hp = ctxE.enter_context(tc.tile_pool(name=f"hE{s}", bufs=1))
            w1 = load_w(wpE, "w1", 8, 2048)
            b1 = load_bias("b1", 16)
            w2 = load_w(wpE, "w2", 16, 1024)
            b2 = load_bias("b2", 8)
            ffnT = pp_ffnT.tile([128, 8, T_SUP], BF16)
            for c in range(4):
                ct = slice(c * NCH, (c + 1) * NCH)
                hT = hp.tile([128, 16, NCH], BF16, tag="hT")
                big_matmul(lambda mi: hT[:, mi, :], w1, 8,
                           lambda k, ct=ct: xn2T[:, k, ct], NCH,
                           func=AF.Gelu, bias_t=b1, bias_key="b1")
                big_matmul(lambda mi, ct=ct: ffnT[:, mi, ct], w2, 16,
                           lambda k: hT[:, k, :], NCH,
                           bias_t=b2, bias_key="b2")
            ctxE.close()
            p_xn2T.close()

            # ================= Phase F: GRU-style fusion ====================
            p_rT = ExitStack()
            pp_rT = p_rT.enter_context(tc.tile_pool(name=f"rT{s}", bufs=1))
            ctxF = ExitStack()
            wpF = ctxF.enter_context(tc.tile_pool(name=f"wF{s}", bufs=1))
            fstg = ctxF.enter_context(tc.tile_pool(name=f"fstg{s}", bufs=2))
            p_x1T = ExitStack()
            pp_x1T = p_x1T.enter_context(tc.tile_pool(name=f"x1T{s}", bufs=1))
            x1T = pp_x1T.tile([128, 8, T_SUP], BF16)
            for tt in range(NT_SUP):
                xb = tokp.tile([128, D], BF16, tag="tokbf")
                nc.gpsimd.dma_start(xb[:], x1_spill[s, tt * 128: tt * 128 + 128, :])
                nc.sync.dma_start_transpose(
                    x1T[:, :, tt * 128:(tt + 1) * 128], xb[:])
            wr = load_w(wpF, "wr", 16, 1024)
            brb = load_bias("br", 8)
            rT = pp_rT.tile([128, 8, T_SUP], BF16)
            for c in range(4):
                ct = slice(c * NCH, (c + 1) * NCH)
                big_matmul(lambda mi, ct=ct: rT[:, mi, ct], wr, 16,
                           lambda k, ct=ct: (x1T if k < 8 else ffnT)[:, k % 8, ct],
                           NCH, func=AF.Sigmoid, bias_t=brb, bias_key="br")
                for m in range(8):
                    nc.vector.tensor_mul(rT[:, m, ct], rT[:, m, ct], x1T[:, m, ct])
            wz = load_w(wpF, "wz", 16, 1024)
            bz = load_bias("bz", 8)
            for c in range(4):
                ct = slice(c * NCH, (c + 1) * NCH)
                zc = fstg.tile([128, 8, NCH], BF16, tag="z480")
                big_matmul(lambda mi: zc[:, mi, :], wz, 16,
                           lambda k, ct=ct: (x1T if k < 8 else ffnT)[:, k % 8, ct],
                           NCH, func=AF.Sigmoid, bias_t=bz, bias_key="bz")
                nc.sync.dma_start(z_spill[s, :, :, ct], zc[:])
            p_x1T.close()
            wc = load_w(wpF, "wc", 16, 1024)
            bc = load_bias("bc", 8)
            for c in range(5):     # FCH chunks (384 = 3 token tiles)
                ct = slice(c * FCH, (c + 1) * FCH)
                hc = fstg.tile([128, 8, FCH], BF16, tag="f384")
                big_matmul(lambda mi: hc[:, mi, :], wc, 16,
                           lambda k, ct=ct: (rT if k < 8 else ffnT)[:, k % 8, ct],
                           FCH, func=AF.Tanh, bias_t=bc, bias_key="bc")
                zc = fstg.tile([128, 8, FCH], BF16, tag="f384")
                nc.sync.dma_start(zc[:], z_spill[s, :, :, ct])
                for i in range(3):
                    tt = c * 3 + i
                    t0 = sT0 + tt * 128
                    ztok = tokp.tile([128, D], BF16, tag="tokbf")
                    httok = tokp.tile([128, D], BF16, tag="tokbf")
                    for f in range(8):
                        nc.sync.dma_start_transpose(
                            ztok[:, f * 128:(f + 1) * 128],
                            zc[:, f, i * 128: i * 128 + 128])
                        nc.sync.dma_start_transpose(
                            httok[:, f * 128:(f + 1) * 128],
                            hc[:, f, i * 128: i * 128 + 128])
                    x1 = tokp.tile([128, D], F32, tag="tokf32")
                    nc.sync.dma_start(x1[:], x1_spill[s, tt * 128: tt * 128 + 128, :])
                    dtt = tokp.tile([128, D], F32, tag="tokf32")
                    nc.vector.tensor_sub(dtt[:], httok[:], x1[:])
                    nc.vector.tensor_mul(dtt[:], dtt[:], ztok[:])
                    nc.vector.tensor_add(dtt[:], x1[:], dtt[:])
                    nc.sync.dma_start(of[t0:t0 + 128, :], dtt[:])
            ctxF.close()
            p_rT.close()
            p_ffnT.close()

    nc.compile()
    return nc


_CACHE = {}


def get_nc(inputs):
    if "nc" not in _CACHE:
        _CACHE["nc"] = build_nc(prep_weights(inputs), ROWS)
    return _CACHE["nc"]


LAST_RESULT = None


def kernel(**inputs):
    global LAST_RESULT
    x = np.asarray(inputs["x"], np.float32)
    nc = get_nc(inputs)
    per = x.shape[0] // NCORES
    in_maps = [{"x": np.ascontiguousarray(x[i * per:(i + 1) * per])}
               for i in range(NCORES)]
    trace = bool(int(os.environ.get("BASSKERNEL_TRACE", "0")))
    res = run_bass_kernel_spmd(nc, in_maps, core_ids=list(range(NCORES)),
                               trace=trace)
    LAST_RESULT = res
    return np.concatenate([np.asarray(r["out"]) for r in res.results], axis=0)



# revision 10
# speedup vs baseline: 1.4145x; 1.4145x over previous
"""Trainium2 Bass kernel for nn_AdvancedTransformerBlock (B=512, L=60, D=1024, H=16).

Data-parallel over batch across 8 NeuronCores (64 rows/core), no collectives.
Per core, activations are feature-major ([feat, tok]) for all matmuls (bf16 in,
fp32 PSUM); token-major only where per-token scalars are needed (LN apply,
residual, final combine); DMA-xbar transposes bridge the two. LN affines and
the 1/sqrt(hd) scale are folded into weights host-side. Attention: S^T via
K^T-stationary matmuls with PE quadrant concurrency (head parity x row
parity), softmax = exp + ones-matmul row-sum broadcast + reciprocal_approx,
O^T via V-stationary matmuls landing directly feature-major.
"""
import os
import sys

for _p in ("/opt/trn_rl_repo", "/root/.axon_site/_ro/trn_rl_repo"):
    if os.path.isdir(_p) and _p not in sys.path:
        sys.path.insert(0, _p)

from contextlib import ExitStack

import ml_dtypes
import numpy as np

import concourse.bass as bass
import concourse.tile as tile
from concourse import bacc, mybir
from concourse.bass_utils import run_bass_kernel_spmd

BF16 = mybir.dt.bfloat16
F32 = mybir.dt.float32
AF = mybir.ActivationFunctionType
OP = mybir.AluOpType

B, L, D = 512, 60, 1024
NH, HD = 16, 64
NCORES = 8
EPS = 1e-5

ROWS = int(os.environ.get("BASSKERNEL_ROWS", B // NCORES))  # rows per core
SUP_ROWS = 32                      # rows per super-chunk
T_SUP = SUP_ROWS * L               # 1920 tokens per super
NT_SUP = T_SUP // 128              # 15 token tiles per super
T_PAD = 1984                       # xn0T free padding (V-proj M=64 overreach)
G_ROWS = 8                         # rows per attention group
T_G = G_ROWS * L                   # 480 tokens per group
NCH = 480                          # big-matmul free chunk (<=512, 4/super)
FCH = 384                          # F-phase streamed chunk (3 token tiles)


def _bf16(a):
    return np.ascontiguousarray(a.astype(ml_dtypes.bfloat16))


def prep_weights(inp):
    """Fold LN affines + qk scale into weights; pre-transpose to lhsT layout
    [in_feat, out_feat]; cast bf16."""
    f32 = {k: np.asarray(v, np.float64) for k, v in inp.items()}
    w = {}

    def fold_ln(W, ln_w, ln_b, b):
        return W * ln_w[None, :], W @ ln_b + b

    for br in ("short", "long"):
        Wf, bf = fold_ln(f32[f"{br}_wqkv"], f32["ln1_w"], f32["ln1_b"],
                         f32[f"{br}_bqkv"])
        scale = 1.0 / np.sqrt(HD)
        Wf = Wf.copy()
        bf = bf.copy()
        Wf[:D] *= scale
        bf[:D] *= scale
        w[f"wqk_{br}"] = _bf16(Wf[: 2 * D].T)      # [1024, 2048]
        w[f"bqk_{br}"] = bf[: 2 * D].astype(np.float32)
        w[f"wv_{br}"] = _bf16(Wf[2 * D:].T)        # [1024, 1024]
        w[f"bv_{br}"] = bf[2 * D:].astype(np.float32)
        w[f"wo_{br}"] = _bf16(f32[f"{br}_wo"].T)   # [1024, 1024]
        w[f"bo_{br}"] = f32[f"{br}_bo"].astype(np.float32)

    w["wg"] = _bf16(f32["gate_w"].T)               # [2048, 1024]
    w["bg"] = f32["gate_b"].astype(np.float32)
    W1, b1 = fold_ln(f32["ffn_w1"], f32["ln2_w"], f32["ln2_b"], f32["ffn_b1"])
    w["w1"] = _bf16(W1.T)                          # [1024, 2048]
    w["b1"] = b1.astype(np.float32)
    w["w2"] = _bf16(f32["ffn_w2"].T)               # [2048, 1024]
    w["b2"] = f32["ffn_b2"].astype(np.float32)
    for k in ("r", "z", "c"):
        w[f"w{k}"] = _bf16(f32[f"{k}_w"].T)        # [2048, 1024]
        w[f"b{k}"] = f32[f"{k}_b"].astype(np.float32)

    # causal mask [128, 16*64] bf16: partition rslot*64+k, free h*64+q: k<=q
    m = np.zeros((128, NH * 64), np.float32)
    tri = (np.arange(L)[:, None] <= np.arange(L)[None, :]).astype(np.float32)
    for rs in range(2):
        for h in range(NH):
            m[rs * 64: rs * 64 + L, h * 64: h * 64 + L] = tri
    w["maskc"] = _bf16(m)
    return w


def build_nc(w, rows):
    supers = rows // SUP_ROWS
    assert rows % SUP_ROWS == 0
    nc = bacc.Bacc("TRN2", target_bir_lowering=False, debug=False,
                   num_devices=NCORES)
    x_ext = nc.declare_dram_parameter("x", [rows, L, D], F32, isOutput=False)
    out_ext = nc.declare_dram_parameter("out", [rows, L, D], F32, isOutput=True)
    xf = x_ext.ap().flatten_outer_dims()       # [rows*L, 1024]
    of = out_ext.ap().flatten_outer_dims()

    wext = {k: nc.inline_tensor(v, name=k) for k, v in w.items()}
    have_bias = {k: bool(np.any(np.asarray(w[k], np.float32)))
                 for k in w if k.startswith("b")}

    so_spill = nc.dram_tensor("so_spill", [supers, 128, 8, T_SUP], BF16)
    lo_spill = nc.dram_tensor("lo_spill", [supers, 128, 8, T_SUP], BF16)
    z_spill = nc.dram_tensor("z_spill", [supers, 5, 128, 3, 8, 128], BF16)
    x1_spill = nc.dram_tensor("x1_spill", [supers, T_SUP, D], F32)

    with tile.TileContext(nc) as tc, ExitStack() as ctx:
        # ----- global pools -----
        consts = ctx.enter_context(tc.tile_pool(name="consts", bufs=1))
        bpool = ctx.enter_context(tc.tile_pool(name="bpool", bufs=2))
        tokp = ctx.enter_context(tc.tile_pool(name="tokp", bufs=4))
        stat = ctx.enter_context(tc.tile_pool(name="stat", bufs=4))
        mm_ps = ctx.enter_context(tc.tile_pool(name="mmps", bufs=2, space="PSUM"))
        st_ps = ctx.enter_context(tc.tile_pool(name="stps", bufs=1, space="PSUM"))
        rs_ps = ctx.enter_context(tc.tile_pool(name="rsps", bufs=1, space="PSUM"))
        ot_ps = ctx.enter_context(tc.tile_pool(name="otps", bufs=1, space="PSUM"))

        mask_t = consts.tile([128, NH * 64], BF16)
        nc.sync.dma_start(mask_t[:], wext["maskc"].ap())
        ones_t = consts.tile([128, 64], BF16)
        nc.vector.memset(ones_t, 1.0)
        eps_t = consts.tile([128, 1], F32)
        nc.vector.memset(eps_t, EPS)

        def load_w(pool, key, kt, mt, tag="wt"):
            t = pool.tile([128, kt, mt], BF16, tag=tag)
            nc.sync.dma_start(t[:], wext[key].ap().rearrange("(k p) m -> p k m", p=128))
            return t

        def load_bias(key, mt):
            t = bpool.tile([128, mt], F32, tag=f"b{mt}")
            nc.sync.dma_start(t[:], wext[key].ap().rearrange("(m p) -> p m", p=128))
            return t

        def evac(i, out_ap, in_ap, func=AF.Copy, bias=None):
            if func == AF.Copy and bias is None:
                if i % 2 == 0:
                    nc.vector.tensor_copy(out_ap, in_ap)
                else:
                    nc.scalar.copy(out_ap, in_ap)
            elif bias is None:
                nc.scalar.activation(out_ap, in_ap, func)
            else:
                nc.scalar.activation(out_ap, in_ap, func, bias=bias)

        def big_matmul(dst_fn, wt, kts, rhs_fn, ntok, func=AF.Copy,
                       bias_t=None, bias_key=None, mtiles=None):
            """for each m-tile: dst_fn(m) [128, ntok] = act(W[:,m].T @ rhs + b)."""
            if mtiles is None:
                mtiles = wt.shape[2] // 128
            for mi in range(mtiles):
                ps = mm_ps.tile([128, 512], F32, tag="mmps")
                for k in range(kts):
                    nc.tensor.matmul(
                        ps[:, 0:ntok],
                        lhsT=wt[:, k, mi * 128:(mi + 1) * 128],
                        rhs=rhs_fn(k),
                        start=(k == 0), stop=(k == kts - 1))
                b = None
                if bias_key is not None and have_bias.get(bias_key, False):
                    b = bias_t[:, mi: mi + 1]
                d = dst_fn(mi)
                src = ps[:, 0:ntok]
                if len(d.shape) == 3:
                    src = src.rearrange("p (a t) -> p a t", t=d.shape[2])
                evac(mi, d, src, func=func, bias=b)

        def ln_stats(xsrc_f32, mvs, tt):
            """bn stats for one token tile -> mvs[:, tt, 0:2] (mean, var)."""
            stt = stat.tile([128, 2, 6], F32, tag="bnst")
            for half in range(2):
                nc.vector.bn_stats(stt[:, half, :],
                                   xsrc_f32[:, half * 512:(half + 1) * 512])
            nc.vector.bn_aggr(mvs[:, tt, :], stt[:])

        def ln_rstd_batch(mvs):
            """mvs[:, :, 1] (var) -> rstd[:, :] for all tiles: 2 ACT ops."""
            rstd = stat.tile([128, NT_SUP], F32, tag="rstdb")
            nc.scalar.activation(rstd[:, :], mvs[:, :, 1], AF.Ln, bias=eps_t[:])
            nc.scalar.activation(rstd[:, :], rstd[:, :], AF.Exp, scale=-0.5)
            return rstd

        def ln_apply(xsrc_f32, dst_bf16, mvs, rstd, tt):
            nc.vector.tensor_scalar(
                out=dst_bf16[:], in0=xsrc_f32[:], scalar1=mvs[:, tt, 0:1],
                scalar2=rstd[:, tt:tt + 1], op0=OP.subtract, op1=OP.mult)

        for s in range(supers):
            sT0 = s * T_SUP
            # ================= Phase A: LN1 + forward transpose =============
            ctxA = ExitStack()
            p_xn0T = ctxA.enter_context(tc.tile_pool(name=f"xn0T{s}", bufs=1))
            xn0T = p_xn0T.tile([128, 8, T_PAD], BF16)
            nc.vector.memset(xn0T[:, :, T_SUP:T_PAD], 0.0)
            mvs = stat.tile([128, NT_SUP, 2], F32, tag="mvsA")
            for tt in range(NT_SUP):
                t0 = sT0 + tt * 128
                xt = tokp.tile([128, D], F32, tag="tokf32")
                nc.sync.dma_start(xt[:], xf[t0:t0 + 128, :])
                ln_stats(xt, mvs, tt)
            rstdA = ln_rstd_batch(mvs)
            for tt in range(NT_SUP):
                t0 = sT0 + tt * 128
                xt = tokp.tile([128, D], F32, tag="tokf32")
                nc.sync.dma_start(xt[:], xf[t0:t0 + 128, :])
                xn = tokp.tile([128, D], BF16, tag="tokbf")
                ln_apply(xt, xn, mvs, rstdA, tt)
                nc.sync.dma_start_transpose(
                    xn0T[:, :, tt * 128:(tt + 1) * 128], xn[:])

            # ================= Phase B: attention branches ==================
            for br in ("short", "long"):
                ctxB = ExitStack()
                wp = ctxB.enter_context(tc.tile_pool(name=f"w{br}{s}", bufs=1))
                gp = ctxB.enter_context(tc.tile_pool(name=f"g{br}{s}", bufs=1))
                p_OT = ctxB.enter_context(tc.tile_pool(name=f"OT{br}{s}", bufs=1))
                attp = ctxB.enter_context(tc.tile_pool(name=f"at{br}{s}", bufs=2))
                bstg = ctxB.enter_context(tc.tile_pool(name=f"bs{br}{s}", bufs=2))
                wqk = load_w(wp, f"wqk_{br}", 8, 2048, tag="wqk")
                wv = load_w(wp, f"wv_{br}", 8, 1024, tag="wvo")
                bqk = load_bias(f"bqk_{br}", 16)
                OT = p_OT.tile([128, 8, T_SUP], BF16)
                for g in range(4):
                    gt0 = g * T_G
                    qkT = gp.tile([128, 16, T_G], BF16, tag="qkT")
                    big_matmul(lambda mi: qkT[:, mi, :], wqk, 8,
                               lambda k: xn0T[:, k, gt0:gt0 + T_G], T_G,
                               bias_t=bqk, bias_key=f"bqk_{br}")
                    Vg = gp.tile([128, 4, 1024], BF16, tag="Vg")
                    for rp in range(4):
                        for c in range(2):
                            vps = mm_ps.tile([128, 512], F32, tag="mmps")
                            for k in range(8):
                                for rsl in range(2):
                                    tok = gt0 + rp * 120 + rsl * 60
                                    nc.tensor.matmul(
                                        vps[rsl * 64: rsl * 64 + 64, :],
                                        lhsT=xn0T[:, k, tok:tok + 64],
                                        rhs=wv[:, k, c * 512:(c + 1) * 512],
                                        start=(k == 0), stop=(k == 7),
                                        skip_group_check=True)
                            nc.vector.tensor_copy(
                                Vg[:, rp, c * 512:(c + 1) * 512], vps[:])
                    for rp in range(4):
                        st = st_ps.tile([128, NH * 64], F32, tag="stps")
                        for rsl in range(2):
                            for h in range(NH):
                                f, par = h // 2, (h % 2) * 64
                                sg = ((h % 2) * 8 + h // 2) * 64
                                tok = rp * 120 + rsl * 60
                                nc.tensor.matmul(
                                    st[rsl * 64: rsl * 64 + L, sg:sg + L],
                                    lhsT=qkT[par:par + HD, 8 + f, tok:tok + L],
                                    rhs=qkT[par:par + HD, f, tok:tok + L],
                                    start=True, stop=True)
                        pt = attp.tile([128, NH, 64], BF16, tag="pt")
                        st3 = st[:, :].rearrange("p (h x) -> p h x", x=64)
                        mt3 = mask_t[:, :].rearrange("p (h x) -> p h x", x=64)
                        # rows 60:64 / 124:128 hold stale-PSUM garbage; every
                        # consumer below reads only the 0:L rows of each slot.
                        nc.scalar.activation(pt[:, :, 0:L], st3[:, :, 0:L], AF.Exp)
                        nc.vector.memset(pt[:, :, L:64], 0.0)
                        if br == "short":
                            nc.vector.tensor_mul(pt[:, :, 0:L], pt[:, :, 0:L],
                                                 mt3[:, :, 0:L])
                        rsum = rs_ps.tile([128, NH * 64], F32, tag="rsps")
                        for rsl in range(2):
                            sl = slice(rsl * 64, rsl * 64 + L)
                            for c in range(2):
                                nc.tensor.matmul(
                                    rsum[rsl * 64: rsl * 64 + 64,
                                         c * 512:(c + 1) * 512],
                                    lhsT=ones_t[sl, 0:64],
                                    rhs=pt[sl, :, :].rearrange("p h x -> p (h x)")[:, c * 512:(c + 1) * 512],
                                    start=True, stop=True)
                        rcp = attp.tile([128, NH, 64], F32, tag="rcp")
                        ptn = attp.tile([128, NH, 64], BF16, tag="ptn")
                        rs3 = rsum[:, :].rearrange("p (h x) -> p h x", x=64)
                        # tracked sliver read orders DVE after the PE row-sum
                        # (the custom-DVE op below is invisible to Tile deps;
                        # DVE itself is in-order, so this fence suffices)
                        nc.vector.tensor_copy(rcp[:, :, 0:1], rs3[:, :, 0:1])
                        nc.vector.reciprocal_approx_fast(rcp[:, :, 0:L], rs3[:, :, 0:L])
                        nc.vector.tensor_mul(ptn[:, :, 0:L], pt[:, :, 0:L], rcp[:, :, 0:L])
                        ot = ot_ps.tile([128, 8 * 128], F32, tag="otps")
                        for rsl in range(2):
                            for h in range(NH):
                                f, hp = h // 2, h % 2
                                sg = (h % 2) * 8 + h // 2
                                nc.tensor.matmul(
                                    ot[hp * 64: hp * 64 + HD,
                                       rsl * 512 + f * 64: rsl * 512 + f * 64 + L],
                                    lhsT=Vg[rsl * 64: rsl * 64 + L, rp, h * HD:(h + 1) * HD],
                                    rhs=ptn[rsl * 64: rsl * 64 + L, sg, 0:L],
                                    start=True, stop=True)
                        for rsl in range(2):
                            tok0 = gt0 + rp * 120 + rsl * 60
                            nc.scalar.copy(
                                OT[:, :, tok0: tok0 + L],
                                ot[:, rsl * 512:(rsl + 1) * 512].rearrange(
                                    "p (f y) -> p f y", y=64)[:, :, 0:L])
                # out projection
                wo = load_w(wp, f"wo_{br}", 8, 1024, tag="wvo")
                bo = load_bias(f"bo_{br}", 8)
                spill = so_spill if br == "short" else lo_spill
                for c in range(4):
                    ct = slice(c * NCH, (c + 1) * NCH)
                    bo_c = bstg.tile([128, 8, NCH], BF16, tag="bstg")
                    big_matmul(lambda mi: bo_c[:, mi, :], wo, 8,
                               lambda k, ct=ct: OT[:, k, ct], NCH,
                               bias_t=bo, bias_key=f"bo_{br}")
                    nc.sync.dma_start(spill[s, :, :, ct], bo_c[:])
                ctxB.close()
            ctxA.close()   # xn0T dead

            # ================= Phase D: gate + combine + LN2 ================
            p_ffnT = ExitStack()
            pp_ffnT = p_ffnT.enter_context(tc.tile_pool(name=f"ffnT{s}", bufs=1))
            p_xn2T = ExitStack()
            pp_xn2T = p_xn2T.enter_context(tc.tile_pool(name=f"xn2T{s}", bufs=1))
            p_attnT = ExitStack()
            pp_attnT = p_attnT.enter_context(tc.tile_pool(name=f"attnT{s}", bufs=1))
            ctxD = ExitStack()
            wpD = ctxD.enter_context(tc.tile_pool(name=f"wD{s}", bufs=1))
            dstg = ctxD.enter_context(tc.tile_pool(name=f"dstg{s}", bufs=2))
            wg = load_w(wpD, "wg", 16, 1024)
            bg = load_bias("bg", 8)
            # attnT block-interleaved: [p, token-tile, feat-slot, tok-in-tile]
            # so the reverse transpose is ONE xbar op per token tile.
            attnT = pp_attnT.tile([128, NT_SUP, 8, 128], BF16)
            for c in range(5):
                ct = slice(c * FCH, (c + 1) * FCH)
                soc = dstg.tile([128, 8, FCH], BF16, tag="soc")
                nc.sync.dma_start(soc[:], so_spill[s, :, :, ct])
                loc = dstg.tile([128, 8, FCH], BF16, tag="loc")
                nc.sync.dma_start(loc[:], lo_spill[s, :, :, ct])
                gT = dstg.tile([128, 8, FCH], BF16, tag="gT")
                big_matmul(lambda mi: gT[:, mi, :], wg, 16,
                           lambda k: (soc if k < 8 else loc)[:, k % 8, :],
                           FCH, func=AF.Sigmoid, bias_t=bg, bias_key="bg")
                for m in range(8):
                    dm = dstg.tile([128, FCH], BF16, tag="dm")
                    nc.gpsimd.tensor_sub(dm[:], soc[:, m, :], loc[:, m, :])
                    nc.vector.tensor_mul(dm[:], dm[:], gT[:, m, :])
                    nc.gpsimd.tensor_add(attnT[:, c * 3:(c + 1) * 3, m, :],
                                         loc[:, m, :].rearrange("p (a t) -> p a t", t=128),
                                         dm[:].rearrange("p (a t) -> p a t", t=128))
            ctxD.close()
            xn2T = pp_xn2T.tile([128, 8, T_SUP], BF16)
            mvs2 = stat.tile([128, NT_SUP, 2], F32, tag="mvsD")
            for tt in range(NT_SUP):
                t0 = sT0 + tt * 128
                atok = tokp.tile([128, D], BF16, tag="tokbf")
                nc.scalar.dma_start_transpose(
                    atok[:, :].rearrange("p (k t) -> p k t", t=128),
                    attnT[:, tt, :, :].rearrange("p a t -> p (a t)"))
                xt = tokp.tile([128, D], F32, tag="tokf32")
                nc.sync.dma_start(xt[:], xf[t0:t0 + 128, :])
                x1 = tokp.tile([128, D], F32, tag="tokf32")
                nc.vector.tensor_add(x1[:], xt[:], atok[:])
                nc.sync.dma_start(x1_spill[s, tt * 128: tt * 128 + 128, :], x1[:])
                ln_stats(x1, mvs2, tt)
            rstdD = ln_rstd_batch(mvs2)
            for tt in range(NT_SUP):
                x1 = tokp.tile([128, D], F32, tag="tokf32")
                nc.sync.dma_start(x1[:], x1_spill[s, tt * 128: tt * 128 + 128, :])
                xn2 = tokp.tile([128, D], BF16, tag="tokbf")
                ln_apply(x1, xn2, mvs2, rstdD, tt)
                nc.sync.dma_start_transpose(
                    xn2T[:, :, tt * 128:(tt + 1) * 128], xn2[:])
            p_attnT.close()

            # ================= Phase E: FFN =================================
            ctxE = ExitStack()
            wpE = ctxE.enter_context(tc.tile_pool(name=f"wE{s}", bufs=2))
            hp = ctxE.enter_context(tc.tile_pool(name=f"hE{s}", bufs=1))
            w1 = load_w(wpE, "w1", 8, 2048)
            b1 = load_bias("b1", 16)
            w2 = load_w(wpE, "w2", 16, 1024)
            b2 = load_bias("b2", 8)
            ffnT = pp_ffnT.tile([128, 8, T_SUP], BF16)
            for c in range(4):
                ct = slice(c * NCH, (c + 1) * NCH)
                hT = hp.tile([128, 16, NCH], BF16, tag="hT")
                big_matmul(lambda mi: hT[:, mi, :], w1, 8,
                           lambda k, ct=ct: xn2T[:, k, ct], NCH,
                           func=AF.Gelu, bias_t=b1, bias_key="b1")
                big_matmul(lambda mi, ct=ct: ffnT[:, mi, ct], w2, 16,
                           lambda k: hT[:, k, :], NCH,
                           bias_t=b2, bias_key="b2")
            ctxE.close()
            p_xn2T.close()

            # ================= Phase F: GRU-style fusion ====================
            p_rT = ExitStack()
            pp_rT = p_rT.enter_context(tc.tile_pool(name=f"rT{s}", bufs=1))
            ctxF = ExitStack()
            wpF = ctxF.enter_context(tc.tile_pool(name=f"wF{s}", bufs=1))
            fstg = ctxF.enter_context(tc.tile_pool(name=f"fstg{s}", bufs=2))
            p_x1T = ExitStack()
            pp_x1T = p_x1T.enter_context(tc.tile_pool(name=f"x1T{s}", bufs=1))
            x1T = pp_x1T.tile([128, 8, T_SUP], BF16)
            for tt in range(NT_SUP):
                xb = tokp.tile([128, D], BF16, tag="tokbf")
                nc.gpsimd.dma_start(xb[:], x1_spill[s, tt * 128: tt * 128 + 128, :])
                nc.sync.dma_start_transpose(
                    x1T[:, :, tt * 128:(tt + 1) * 128], xb[:])
            wr = load_w(wpF, "wr", 16, 1024)
            brb = load_bias("br", 8)
            rT = pp_rT.tile([128, 8, T_SUP], BF16)
            for c in range(4):
                ct = slice(c * NCH, (c + 1) * NCH)
                big_matmul(lambda mi, ct=ct: rT[:, mi, ct], wr, 16,
                           lambda k, ct=ct: (x1T if k < 8 else ffnT)[:, k % 8, ct],
                           NCH, func=AF.Sigmoid, bias_t=brb, bias_key="br")
                for m in range(8):
                    nc.vector.tensor_mul(rT[:, m, ct], rT[:, m, ct], x1T[:, m, ct])
            wz = load_w(wpF, "wz", 16, 1024)
            bz = load_bias("bz", 8)
            for c in range(5):
                ct = slice(c * FCH, (c + 1) * FCH)
                zc = fstg.tile([128, 3, 8, 128], BF16, tag="z384")
                big_matmul(lambda mi: zc[:, :, mi, :], wz, 16,
                           lambda k, ct=ct: (x1T if k < 8 else ffnT)[:, k % 8, ct],
                           FCH, func=AF.Sigmoid, bias_t=bz, bias_key="bz")
                nc.sync.dma_start(z_spill[s, c], zc[:])
            p_x1T.close()
            wc = load_w(wpF, "wc", 16, 1024)
            bc = load_bias("bc", 8)
            for c in range(5):     # FCH chunks (384 = 3 token tiles)
                ct = slice(c * FCH, (c + 1) * FCH)
                hc = fstg.tile([128, 3, 8, 128], BF16, tag="f384")
                big_matmul(lambda mi: hc[:, :, mi, :], wc, 16,
                           lambda k, ct=ct: (rT if k < 8 else ffnT)[:, k % 8, ct],
                           FCH, func=AF.Tanh, bias_t=bc, bias_key="bc")
                zc = fstg.tile([128, 3, 8, 128], BF16, tag="f384")
                nc.sync.dma_start(zc[:], z_spill[s, c])
                for i in range(3):
                    tt = c * 3 + i
                    t0 = sT0 + tt * 128
                    ztok = tokp.tile([128, D], BF16, tag="tokbf")
                    httok = tokp.tile([128, D], BF16, tag="tokbf")
                    nc.sync.dma_start_transpose(
                        ztok[:, :].rearrange("p (k t) -> p k t", t=128),
                        zc[:, i, :, :].rearrange("p k t -> p (k t)"))
                    nc.scalar.dma_start_transpose(
                        httok[:, :].rearrange("p (k t) -> p k t", t=128),
                        hc[:, i, :, :].rearrange("p k t -> p (k t)"))
                    x1 = tokp.tile([128, D], F32, tag="tokf32")
                    nc.sync.dma_start(x1[:], x1_spill[s, tt * 128: tt * 128 + 128, :])
                    dtt = tokp.tile([128, D], F32, tag="tokf32")
                    nc.gpsimd.tensor_sub(dtt[:], httok[:], x1[:])
                    nc.vector.tensor_mul(dtt[:], dtt[:], ztok[:])
                    nc.gpsimd.tensor_add(dtt[:], x1[:], dtt[:])
                    nc.sync.dma_start(of[t0:t0 + 128, :], dtt[:])
            ctxF.close()
            p_rT.close()
            p_ffnT.close()

    nc.compile()
    return nc


_CACHE = {}


def get_nc(inputs):
    if "nc" not in _CACHE:
        _CACHE["nc"] = build_nc(prep_weights(inputs), ROWS)
    return _CACHE["nc"]


LAST_RESULT = None


def kernel(**inputs):
    global LAST_RESULT
    x = np.asarray(inputs["x"], np.float32)
    nc = get_nc(inputs)
    per = x.shape[0] // NCORES
    in_maps = [{"x": np.ascontiguousarray(x[i * per:(i + 1) * per])}
               for i in range(NCORES)]
    trace = bool(int(os.environ.get("BASSKERNEL_TRACE", "0")))
    res = run_bass_kernel_spmd(nc, in_maps, core_ids=list(range(NCORES)),
                               trace=trace)
    LAST_RESULT = res
    return np.concatenate([np.asarray(r["out"]) for r in res.results], axis=0)



# revision 27
# speedup vs baseline: 1.6835x; 1.1902x over previous
"""Trainium2 Bass kernel for nn_AdvancedTransformerBlock (B=512, L=60, D=1024, H=16).

Data-parallel over batch across 8 NeuronCores (64 rows/core), no collectives.
Per core, activations are feature-major ([feat, tok]) for all matmuls (bf16 in,
fp32 PSUM); token-major only where per-token scalars are needed (LN apply,
residual, final combine); DMA-xbar transposes bridge the two. LN affines and
the 1/sqrt(hd) scale are folded into weights host-side. Attention: S^T via
K^T-stationary matmuls with PE quadrant concurrency (head parity x row
parity), softmax = exp + ones-matmul row-sum broadcast + reciprocal_approx,
O^T via V-stationary matmuls landing directly feature-major.
"""
import os
import sys

for _p in ("/opt/trn_rl_repo", "/root/.axon_site/_ro/trn_rl_repo"):
    if os.path.isdir(_p) and _p not in sys.path:
        sys.path.insert(0, _p)

from contextlib import ExitStack

import ml_dtypes
import numpy as np

import concourse.bass as bass
import concourse.tile as tile
from concourse import bacc, mybir
from concourse.bass_utils import run_bass_kernel_spmd

BF16 = mybir.dt.bfloat16
FP8 = mybir.dt.float8e4
F32 = mybir.dt.float32
AF = mybir.ActivationFunctionType
OP = mybir.AluOpType
DROW = mybir.MatmulPerfMode.DoubleRow

B, L, D = 512, 60, 1024
NH, HD = 16, 64
NCORES = 8
EPS = 1e-5

ROWS = int(os.environ.get("BASSKERNEL_ROWS", B // NCORES))  # rows per core
SUP_ROWS = 32                      # rows per super-chunk
T_SUP = SUP_ROWS * L               # 1920 tokens per super
NT_SUP = T_SUP // 128              # 15 token tiles per super
T_PAD = 1984                       # xn0T free padding (V-proj M=64 overreach)
G_ROWS = 8                         # rows per attention group
T_G = G_ROWS * L                   # 480 tokens per group
NCH = 480                          # big-matmul free chunk (<=512, 4/super)
FCH = 384                          # F-phase streamed chunk (3 token tiles)


def _bf16(a):
    return np.ascontiguousarray(a.astype(ml_dtypes.bfloat16))


WSCALE = 64.0   # fp8 weights pre-scaled x64 (avoids e4m3 subnormals); evac undoes it


def _fp8(a):
    return np.ascontiguousarray(
        (np.asarray(a, np.float32) * WSCALE).astype(ml_dtypes.float8_e4m3))


def prep_weights(inp):
    """Fold LN affines + qk scale into weights; pre-transpose to lhsT layout
    [in_feat, out_feat]; cast bf16."""
    f32 = {k: np.asarray(v, np.float64) for k, v in inp.items()}
    w = {}

    def fold_ln(W, ln_w, ln_b, b):
        return W * ln_w[None, :], W @ ln_b + b

    for br in ("short", "long"):
        Wf, bf = fold_ln(f32[f"{br}_wqkv"], f32["ln1_w"], f32["ln1_b"],
                         f32[f"{br}_bqkv"])
        scale = 1.0 / np.sqrt(HD)
        Wf = Wf.copy()
        bf = bf.copy()
        Wf[:D] *= scale
        bf[:D] *= scale
        w[f"wqk_{br}"] = _fp8(Wf[: 2 * D].T)       # [1024, 2048]
        w[f"bqk_{br}"] = bf[: 2 * D].astype(np.float32)
        w[f"wv_{br}"] = _fp8(Wf[2 * D:].T)         # [1024, 1024]
        w[f"bv_{br}"] = bf[2 * D:].astype(np.float32)
        w[f"wo_{br}"] = _fp8(f32[f"{br}_wo"].T)    # [1024, 1024]
        w[f"bo_{br}"] = f32[f"{br}_bo"].astype(np.float32)

    w["wg"] = _fp8(f32["gate_w"].T)                # [2048, 1024]
    w["bg"] = f32["gate_b"].astype(np.float32)
    W1, b1 = fold_ln(f32["ffn_w1"], f32["ln2_w"], f32["ln2_b"], f32["ffn_b1"])
    w["w1"] = _fp8(W1.T)                           # [1024, 2048]
    w["b1"] = b1.astype(np.float32)
    w["w2"] = _bf16(f32["ffn_w2"].T)               # [2048, 1024]
    w["b2"] = f32["ffn_b2"].astype(np.float32)
    for k in ("r", "z", "c"):
        dt = _fp8 if k == "r" else _bf16           # z/c too error-sensitive
        w[f"w{k}"] = dt(f32[f"{k}_w"].T)           # [2048, 1024]
        w[f"b{k}"] = f32[f"{k}_b"].astype(np.float32)

    # causal mask [128, 16*64] bf16: partition rslot*64+k, free h*64+q: k<=q
    m = np.zeros((128, NH * 64), np.float32)
    tri = (np.arange(L)[:, None] <= np.arange(L)[None, :]).astype(np.float32)
    for rs in range(2):
        for h in range(NH):
            m[rs * 64: rs * 64 + L, h * 64: h * 64 + L] = tri
    w["maskc"] = _bf16(m)
    return w


def build_nc(w, rows):
    supers = rows // SUP_ROWS
    assert rows % SUP_ROWS == 0
    nc = bacc.Bacc("TRN2", target_bir_lowering=False, debug=False,
                   num_devices=NCORES)
    x_ext = nc.declare_dram_parameter("x", [rows, L, D], F32, isOutput=False)
    out_ext = nc.declare_dram_parameter("out", [rows, L, D], F32, isOutput=True)
    xf = x_ext.ap().flatten_outer_dims()       # [rows*L, 1024]
    of = out_ext.ap().flatten_outer_dims()

    wext = {k: nc.inline_tensor(v, name=k) for k, v in w.items()}
    have_bias = {k: bool(np.any(np.asarray(w[k], np.float32)))
                 for k in w if k.startswith("b")}

    so_spill = nc.dram_tensor("so_spill", [supers, 128, 8, T_SUP], BF16)
    lo_spill = nc.dram_tensor("lo_spill", [supers, 128, 8, T_SUP], BF16)
    z_spill = nc.dram_tensor("z_spill", [supers, 5, 128, 3, 8, 128], BF16)
    x1_spill = nc.dram_tensor("x1_spill", [supers, T_SUP, D], F32)

    with tile.TileContext(nc) as tc, ExitStack() as ctx:
        # ----- global pools -----
        consts = ctx.enter_context(tc.tile_pool(name="consts", bufs=1))
        bpool = ctx.enter_context(tc.tile_pool(name="bpool", bufs=2))
        tokp = ctx.enter_context(tc.tile_pool(name="tokp", bufs=4))
        stat = ctx.enter_context(tc.tile_pool(name="stat", bufs=4))
        mm_ps = ctx.enter_context(tc.tile_pool(name="mmps", bufs=2, space="PSUM"))
        st_ps = ctx.enter_context(tc.tile_pool(name="stps", bufs=1, space="PSUM"))
        rs_ps = ctx.enter_context(tc.tile_pool(name="rsps", bufs=1, space="PSUM"))
        ot_ps = ctx.enter_context(tc.tile_pool(name="otps", bufs=1, space="PSUM"))

        mask_t = consts.tile([128, NH * 64], BF16)
        nc.sync.dma_start(mask_t[:], wext["maskc"].ap())
        ones_t = consts.tile([128, 64], BF16)
        nc.vector.memset(ones_t, 1.0)
        eps_t = consts.tile([128, 1], F32)
        nc.vector.memset(eps_t, EPS)

        def load_w(pool, key, kt, mt, tag="wt", dt=BF16):
            t = pool.tile([128, kt, mt], dt, tag=tag)
            nc.sync.dma_start(t[:], wext[key].ap().rearrange("(k p) m -> p k m", p=128))
            return t

        def load_bias(key, mt):
            t = bpool.tile([128, mt], F32, tag=f"b{mt}")
            nc.sync.dma_start(t[:], wext[key].ap().rearrange("(m p) -> p m", p=128))
            return t

        def evac(i, out_ap, in_ap, func=AF.Copy, bias=None, scale=1.0):
            if func == AF.Copy and bias is None and scale == 1.0:
                if i % 2 == 0:
                    nc.vector.tensor_copy(out_ap, in_ap)
                else:
                    nc.scalar.copy(out_ap, in_ap)
            elif func == AF.Copy and bias is None and i % 2 == 0:
                nc.vector.tensor_scalar(out=out_ap, in0=in_ap, scalar1=scale,
                                        scalar2=None, op0=OP.mult)
            elif bias is None:
                nc.scalar.activation(out_ap, in_ap, func, scale=scale)
            else:
                nc.scalar.activation(out_ap, in_ap, func, bias=bias, scale=scale)

        def big_matmul(dst_fn, wt, kts, rhs_fn, ntok, func=AF.Copy,
                       bias_t=None, bias_key=None, mtiles=None, fp8=False):
            """for each m-tile: dst_fn(m) [128, ntok] = act(W[:,m].T @ rhs + b).
            fp8: wt is fp8 x WSCALE, rhs_fn(g) returns a [128, 2, ntok] k-pair
            AP; runs DoubleRow matmuls and descales at evac."""
            if mtiles is None:
                mtiles = wt.shape[2] // 128
            for mi in range(mtiles):
                ps = mm_ps.tile([128, 512], F32, tag="mmps")
                if fp8:
                    half = ntok // 2
                    for n0 in (0, half):
                        for g in range(kts // 2):
                            nc.tensor.matmul(
                                ps[:, n0:n0 + half],
                                lhsT=wt[:, 2 * g:2 * g + 2, mi * 128:(mi + 1) * 128],
                                rhs=rhs_fn(g)[:, :, n0:n0 + half],
                                start=(g == 0), stop=(g == kts // 2 - 1),
                                perf_mode=DROW)
                else:
                    for k in range(kts):
                        nc.tensor.matmul(
                            ps[:, 0:ntok],
                            lhsT=wt[:, k, mi * 128:(mi + 1) * 128],
                            rhs=rhs_fn(k),
                            start=(k == 0), stop=(k == kts - 1))
                b = None
                if bias_key is not None and have_bias.get(bias_key, False):
                    b = bias_t[:, mi: mi + 1]
                d = dst_fn(mi)
                src = ps[:, 0:ntok]
                if len(d.shape) == 3:
                    src = src.rearrange("p (a t) -> p a t", t=d.shape[2])
                evac(mi, d, src, func=func, bias=b,
                     scale=(1.0 / WSCALE) if fp8 else 1.0)

        def ln_stats(xsrc_f32, mvs, tt):
            """bn stats for one token tile -> mvs[:, tt, 0:2] (mean, var)."""
            stt = stat.tile([128, 2, 6], F32, tag="bnst")
            for half in range(2):
                nc.vector.bn_stats(stt[:, half, :],
                                   xsrc_f32[:, half * 512:(half + 1) * 512])
            nc.vector.bn_aggr(mvs[:, tt, :], stt[:])

        def ln_rstd_batch(mvs):
            """mvs[:, :, 1] (var) -> rstd[:, :] for all tiles: 2 ACT ops."""
            rstd = stat.tile([128, NT_SUP], F32, tag="rstdb")
            nc.scalar.activation(rstd[:, :], mvs[:, :, 1], AF.Ln, bias=eps_t[:])
            nc.scalar.activation(rstd[:, :], rstd[:, :], AF.Exp, scale=-0.5)
            return rstd

        def ln_apply(xsrc_f32, dst_bf16, mvs, rstd, tt):
            nc.vector.tensor_scalar(
                out=dst_bf16[:], in0=xsrc_f32[:], scalar1=mvs[:, tt, 0:1],
                scalar2=rstd[:, tt:tt + 1], op0=OP.subtract, op1=OP.mult)

        for s in range(supers):
            sT0 = s * T_SUP
            # ================= Phase A: LN1 + forward transpose =============
            ctxA = ExitStack()
            p_xn8 = ctxA.enter_context(tc.tile_pool(name=f"xn8{s}", bufs=1))
            xn8 = p_xn8.tile([128, 8, T_PAD], FP8)
            nc.vector.memset(xn8[:, :, T_SUP:T_PAD], 0.0)
            ctxAb = ExitStack()
            p_xnb = ctxAb.enter_context(tc.tile_pool(name=f"xnb{s}", bufs=1))
            xn0T = p_xnb.tile([128, 8, T_SUP], BF16)
            mvs = stat.tile([128, NT_SUP, 2], F32, tag="mvsA")
            for tt in range(NT_SUP):
                t0 = sT0 + tt * 128
                xt = tokp.tile([128, D], F32, tag="tokf32")
                nc.sync.dma_start(xt[:], xf[t0:t0 + 128, :])
                ln_stats(xt, mvs, tt)
            rstdA = ln_rstd_batch(mvs)
            for tt in range(NT_SUP):
                t0 = sT0 + tt * 128
                xt = tokp.tile([128, D], F32, tag="tokf32")
                nc.sync.dma_start(xt[:], xf[t0:t0 + 128, :])
                xn = tokp.tile([128, D], BF16, tag="tokbf")
                ln_apply(xt, xn, mvs, rstdA, tt)
                nc.sync.dma_start_transpose(
                    xn0T[:, :, tt * 128:(tt + 1) * 128], xn[:])
                nc.vector.tensor_copy(
                    xn8[:, :, tt * 128:(tt + 1) * 128],
                    xn0T[:, :, tt * 128:(tt + 1) * 128])
            ctxAb.close()

            # ================= Phase B: attention branches ==================
            for br in ("short", "long"):
                ctxB = ExitStack()
                wp = ctxB.enter_context(tc.tile_pool(name=f"w{br}{s}", bufs=1))
                gp = ctxB.enter_context(tc.tile_pool(name=f"g{br}{s}", bufs=1))
                p_OT = ctxB.enter_context(tc.tile_pool(name=f"OT{br}{s}", bufs=1))
                attp = ctxB.enter_context(tc.tile_pool(name=f"at{br}{s}", bufs=2))
                bstg = ctxB.enter_context(tc.tile_pool(name=f"bs{br}{s}", bufs=2))
                wqk = load_w(wp, f"wqk_{br}", 8, 2048, tag="wqk", dt=FP8)
                wv = load_w(wp, f"wv_{br}", 8, 1024, tag="wvo", dt=FP8)
                bqk = load_bias(f"bqk_{br}", 16)
                OT = p_OT.tile([128, 8, T_SUP], FP8)
                for g in range(4):
                    gt0 = g * T_G
                    qkT = gp.tile([128, 16, T_G], BF16, tag="qkT")
                    big_matmul(lambda mi: qkT[:, mi, :], wqk, 8,
                               lambda kg: xn8[:, 2 * kg:2 * kg + 2, gt0:gt0 + T_G],
                               T_G, bias_t=bqk, bias_key=f"bqk_{br}", fp8=True)
                    Vg = gp.tile([128, 4, 1024], BF16, tag="Vg")
                    for rp in range(4):
                        for c in range(2):
                            # DoubleRow needs dst base partition 0 (walrus
                            # s3d3_mm_valid_dst_partition), so the rsl=1
                            # quadrant forces plain-mode fp8 here.
                            vps = mm_ps.tile([128, 512], F32, tag="mmps")
                            for k in range(8):
                                for rsl in range(2):
                                    tok = gt0 + rp * 120 + rsl * 60
                                    nc.tensor.matmul(
                                        vps[rsl * 64: rsl * 64 + 64, :],
                                        lhsT=xn8[:, k, tok:tok + 64],
                                        rhs=wv[:, k, c * 512:(c + 1) * 512],
                                        start=(k == 0), stop=(k == 7),
                                        skip_group_check=True)
                            evac(rp * 2 + c, Vg[:, rp, c * 512:(c + 1) * 512],
                                 vps[:], scale=1.0 / WSCALE)
                    for rp in range(4):
                        st = st_ps.tile([128, NH * 64], F32, tag="stps")
                        for rsl in range(2):
                            for h in range(NH):
                                f, par = h // 2, (h % 2) * 64
                                sg = ((h % 2) * 8 + h // 2) * 64
                                tok = rp * 120 + rsl * 60
                                nc.tensor.matmul(
                                    st[rsl * 64: rsl * 64 + L, sg:sg + L],
                                    lhsT=qkT[par:par + HD, 8 + f, tok:tok + L],
                                    rhs=qkT[par:par + HD, f, tok:tok + L],
                                    start=True, stop=True)
                        pt = attp.tile([128, NH, 64], BF16, tag="pt")
                        st3 = st[:, :].rearrange("p (h x) -> p h x", x=64)
                        mt3 = mask_t[:, :].rearrange("p (h x) -> p h x", x=64)
                        # rows 60:64 / 124:128 hold stale-PSUM garbage; every
                        # consumer below reads only the 0:L rows of each slot.
                        nc.scalar.activation(pt[:, :, 0:L], st3[:, :, 0:L], AF.Exp)
                        nc.vector.memset(pt[:, :, L:64], 0.0)
                        if br == "short":
                            nc.vector.tensor_mul(pt[:, :, 0:L], pt[:, :, 0:L],
                                                 mt3[:, :, 0:L])
                        rsum = rs_ps.tile([128, NH * 64], F32, tag="rsps")
                        for rsl in range(2):
                            sl = slice(rsl * 64, rsl * 64 + L)
                            for c in range(2):
                                nc.tensor.matmul(
                                    rsum[rsl * 64: rsl * 64 + 64,
                                         c * 512:(c + 1) * 512],
                                    lhsT=ones_t[sl, 0:64],
                                    rhs=pt[sl, :, :].rearrange("p h x -> p (h x)")[:, c * 512:(c + 1) * 512],
                                    start=True, stop=True)
                        rcp = attp.tile([128, NH, 64], F32, tag="rcp")
                        ptn = attp.tile([128, NH, 64], BF16, tag="ptn")
                        rs3 = rsum[:, :].rearrange("p (h x) -> p h x", x=64)
                        # tracked sliver read orders DVE after the PE row-sum
                        # (the custom-DVE op below is invisible to Tile deps;
                        # DVE itself is in-order, so this fence suffices)
                        nc.vector.tensor_copy(rcp[:, :, 0:1], rs3[:, :, 0:1])
                        nc.vector.reciprocal_approx_fast(rcp[:, :, 0:L], rs3[:, :, 0:L])
                        nc.vector.tensor_mul(ptn[:, :, 0:L], pt[:, :, 0:L], rcp[:, :, 0:L])
                        ot = ot_ps.tile([128, 8 * 128], F32, tag="otps")
                        for rsl in range(2):
                            for h in range(NH):
                                f, hp = h // 2, h % 2
                                sg = (h % 2) * 8 + h // 2
                                nc.tensor.matmul(
                                    ot[hp * 64: hp * 64 + HD,
                                       rsl * 512 + f * 64: rsl * 512 + f * 64 + L],
                                    lhsT=Vg[rsl * 64: rsl * 64 + L, rp, h * HD:(h + 1) * HD],
                                    rhs=ptn[rsl * 64: rsl * 64 + L, sg, 0:L],
                                    start=True, stop=True)
                        for rsl in range(2):
                            tok0 = gt0 + rp * 120 + rsl * 60
                            nc.scalar.copy(
                                OT[:, :, tok0: tok0 + L],
                                ot[:, rsl * 512:(rsl + 1) * 512].rearrange(
                                    "p (f y) -> p f y", y=64)[:, :, 0:L])
                # out projection
                wo = load_w(wp, f"wo_{br}", 8, 1024, tag="wvo", dt=FP8)
                bo = load_bias(f"bo_{br}", 8)
                spill = so_spill if br == "short" else lo_spill
                for c in range(4):
                    ct = slice(c * NCH, (c + 1) * NCH)
                    bo_c = bstg.tile([128, 8, NCH], BF16, tag="bstg")
                    big_matmul(lambda mi: bo_c[:, mi, :], wo, 8,
                               lambda kg, ct=ct: OT[:, 2 * kg:2 * kg + 2, ct], NCH,
                               bias_t=bo, bias_key=f"bo_{br}", fp8=True)
                    nc.sync.dma_start(spill[s, :, :, ct], bo_c[:])
                ctxB.close()
            ctxA.close()   # xn0T dead

            # ================= Phase D: gate + combine + LN2 ================
            p_ffnT = ExitStack()
            pp_ffnT = p_ffnT.enter_context(tc.tile_pool(name=f"ffnT{s}", bufs=1))
            p_xn2T = ExitStack()
            pp_xn2T = p_xn2T.enter_context(tc.tile_pool(name=f"xn2T{s}", bufs=1))
            p_attnT = ExitStack()
            pp_attnT = p_attnT.enter_context(tc.tile_pool(name=f"attnT{s}", bufs=1))
            ctxD = ExitStack()
            wpD = ctxD.enter_context(tc.tile_pool(name=f"wD{s}", bufs=1))
            dstg = ctxD.enter_context(tc.tile_pool(name=f"dstg{s}", bufs=2))
            wg = load_w(wpD, "wg", 16, 1024, dt=FP8)
            bg = load_bias("bg", 8)
            # attnT block-interleaved: [p, token-tile, feat-slot, tok-in-tile]
            # so the reverse transpose is ONE xbar op per token tile.
            attnT = pp_attnT.tile([128, NT_SUP, 8, 128], BF16)
            for c in range(5):
                ct = slice(c * FCH, (c + 1) * FCH)
                soc = dstg.tile([128, 8, FCH], BF16, tag="soc")
                nc.sync.dma_start(soc[:], so_spill[s, :, :, ct])
                loc = dstg.tile([128, 8, FCH], BF16, tag="loc")
                nc.sync.dma_start(loc[:], lo_spill[s, :, :, ct])
                soc8 = dstg.tile([128, 8, FCH], FP8, tag="soc8")
                nc.vector.tensor_copy(soc8[:], soc[:])
                loc8 = dstg.tile([128, 8, FCH], FP8, tag="loc8")
                nc.scalar.copy(loc8[:], loc[:])
                gT = dstg.tile([128, 8, FCH], BF16, tag="gT")
                big_matmul(lambda mi: gT[:, mi, :], wg, 16,
                           lambda kg: (soc8 if kg < 4 else loc8)[:, 2 * (kg % 4):2 * (kg % 4) + 2, :],
                           FCH, func=AF.Sigmoid, bias_t=bg, bias_key="bg", fp8=True)
                for m in range(8):
                    dm = dstg.tile([128, FCH], BF16, tag="dm")
                    nc.gpsimd.tensor_sub(dm[:], soc[:, m, :], loc[:, m, :])
                    nc.vector.tensor_mul(dm[:], dm[:], gT[:, m, :])
                    nc.gpsimd.tensor_add(attnT[:, c * 3:(c + 1) * 3, m, :],
                                         loc[:, m, :].rearrange("p (a t) -> p a t", t=128),
                                         dm[:].rearrange("p (a t) -> p a t", t=128))
            ctxD.close()
            xn2T8 = pp_xn2T.tile([128, 8, T_SUP], FP8, tag="xn2T8")
            mvs2 = stat.tile([128, NT_SUP, 2], F32, tag="mvsD")
            for tt in range(NT_SUP):
                t0 = sT0 + tt * 128
                atok = tokp.tile([128, D], BF16, tag="tokbf")
                nc.scalar.dma_start_transpose(
                    atok[:, :].rearrange("p (k t) -> p k t", t=128),
                    attnT[:, tt, :, :].rearrange("p a t -> p (a t)"))
                xt = tokp.tile([128, D], F32, tag="tokf32")
                nc.sync.dma_start(xt[:], xf[t0:t0 + 128, :])
                x1 = tokp.tile([128, D], F32, tag="tokf32")
                nc.vector.tensor_add(x1[:], xt[:], atok[:])
                nc.sync.dma_start(x1_spill[s, tt * 128: tt * 128 + 128, :], x1[:])
                ln_stats(x1, mvs2, tt)
            rstdD = ln_rstd_batch(mvs2)
            for tt in range(NT_SUP):
                x1 = tokp.tile([128, D], F32, tag="tokf32")
                nc.sync.dma_start(x1[:], x1_spill[s, tt * 128: tt * 128 + 128, :])
                xn2 = tokp.tile([128, D], BF16, tag="tokbf")
                ln_apply(x1, xn2, mvs2, rstdD, tt)
                xnt = tokp.tile([128, D], BF16, tag="tokbf")
                nc.sync.dma_start_transpose(
                    xnt[:, :].rearrange("p (k t) -> p k t", t=128), xn2[:])
                nc.vector.tensor_copy(
                    xn2T8[:, :, tt * 128:(tt + 1) * 128],
                    xnt[:, :].rearrange("p (k t) -> p k t", t=128))
            p_attnT.close()

            # ================= Phase E: FFN =================================
            ctxE = ExitStack()
            wpE = ctxE.enter_context(tc.tile_pool(name=f"wE{s}", bufs=2))
            hp = ctxE.enter_context(tc.tile_pool(name=f"hE{s}", bufs=1))
            w1 = load_w(wpE, "w1", 8, 2048, dt=FP8)
            b1 = load_bias("b1", 16)
            w2 = load_w(wpE, "w2", 16, 1024)
            b2 = load_bias("b2", 8)
            ffnT = pp_ffnT.tile([128, 8, T_SUP], BF16)
            for c in range(4):
                ct = slice(c * NCH, (c + 1) * NCH)
                hT = hp.tile([128, 16, NCH], BF16, tag="hT")
                big_matmul(lambda mi: hT[:, mi, :], w1, 8,
                           lambda kg, ct=ct: xn2T8[:, 2 * kg:2 * kg + 2, ct], NCH,
                           func=AF.Gelu, bias_t=b1, bias_key="b1", fp8=True)
                big_matmul(lambda mi, ct=ct: ffnT[:, mi, ct], w2, 16,
                           lambda k: hT[:, k, :], NCH,
                           bias_t=b2, bias_key="b2")
            ctxE.close()
            p_xn2T.close()

            # ================= Phase F: GRU-style fusion ====================
            p_rT = ExitStack()
            pp_rT = p_rT.enter_context(tc.tile_pool(name=f"rT{s}", bufs=1))
            ctxF = ExitStack()
            wpF = ctxF.enter_context(tc.tile_pool(name=f"wF{s}", bufs=1))
            fstg = ctxF.enter_context(tc.tile_pool(name=f"fstg{s}", bufs=2))
            p_x1T = ExitStack()
            pp_x1T = p_x1T.enter_context(tc.tile_pool(name=f"x1T{s}", bufs=1))
            x1T = pp_x1T.tile([128, 8, T_SUP], BF16)
            x1T8 = pp_x1T.tile([128, 8, T_SUP], FP8, tag="x1T8")
            for tt in range(NT_SUP):
                xb = tokp.tile([128, D], BF16, tag="tokbf")
                nc.gpsimd.dma_start(xb[:], x1_spill[s, tt * 128: tt * 128 + 128, :])
                nc.sync.dma_start_transpose(
                    x1T[:, :, tt * 128:(tt + 1) * 128], xb[:])
                nc.vector.tensor_copy(
                    x1T8[:, :, tt * 128:(tt + 1) * 128],
                    x1T[:, :, tt * 128:(tt + 1) * 128])
            ffnT8 = pp_ffnT.tile([128, 8, T_SUP], FP8, tag="ffnT8")
            for q in range(4):
                nc.scalar.copy(ffnT8[:, 2 * q:2 * q + 2, :], ffnT[:, 2 * q:2 * q + 2, :])
            wr = load_w(wpF, "wr", 16, 1024, dt=FP8)
            brb = load_bias("br", 8)
            rT = pp_rT.tile([128, 8, T_SUP], BF16)
            for c in range(4):
                ct = slice(c * NCH, (c + 1) * NCH)
                big_matmul(lambda mi, ct=ct: rT[:, mi, ct], wr, 16,
                           lambda kg, ct=ct: (x1T8 if kg < 4 else ffnT8)[:, 2 * (kg % 4):2 * (kg % 4) + 2, ct],
                           NCH, func=AF.Sigmoid, bias_t=brb, bias_key="br", fp8=True)
                for m in range(8):
                    nc.vector.tensor_mul(rT[:, m, ct], rT[:, m, ct], x1T[:, m, ct])
            wz = load_w(wpF, "wz", 16, 1024)
            bz = load_bias("bz", 8)
            for c in range(5):
                ct = slice(c * FCH, (c + 1) * FCH)
                zc = fstg.tile([128, 3, 8, 128], BF16, tag="z384")
                big_matmul(lambda mi: zc[:, :, mi, :], wz, 16,
                           lambda k, ct=ct: (x1T if k < 8 else ffnT)[:, k % 8, ct],
                           FCH, func=AF.Sigmoid, bias_t=bz, bias_key="bz")
                nc.sync.dma_start(z_spill[s, c], zc[:])
            p_x1T.close()
            wc = load_w(wpF, "wc", 16, 1024)
            bc = load_bias("bc", 8)
            for c in range(5):     # FCH chunks (384 = 3 token tiles)
                ct = slice(c * FCH, (c + 1) * FCH)
                hc = fstg.tile([128, 3, 8, 128], BF16, tag="f384")
                big_matmul(lambda mi: hc[:, :, mi, :], wc, 16,
                           lambda k, ct=ct: (rT if k < 8 else ffnT)[:, k % 8, ct],
                           FCH, func=AF.Tanh, bias_t=bc, bias_key="bc")
                zc = fstg.tile([128, 3, 8, 128], BF16, tag="f384")
                nc.sync.dma_start(zc[:], z_spill[s, c])
                for i in range(3):
                    tt = c * 3 + i
                    t0 = sT0 + tt * 128
                    ztok = tokp.tile([128, D], BF16, tag="tokbf")
                    httok = tokp.tile([128, D], BF16, tag="tokbf")
                    nc.sync.dma_start_transpose(
                        ztok[:, :].rearrange("p (k t) -> p k t", t=128),
                        zc[:, i, :, :].rearrange("p k t -> p (k t)"))
                    nc.scalar.dma_start_transpose(
                        httok[:, :].rearrange("p (k t) -> p k t", t=128),
                        hc[:, i, :, :].rearrange("p k t -> p (k t)"))
                    x1 = tokp.tile([128, D], F32, tag="tokf32")
                    nc.sync.dma_start(x1[:], x1_spill[s, tt * 128: tt * 128 + 128, :])
                    dtt = tokp.tile([128, D], F32, tag="tokf32")
                    nc.gpsimd.tensor_sub(dtt[:], httok[:], x1[:])
                    nc.vector.tensor_mul(dtt[:], dtt[:], ztok[:])
                    nc.gpsimd.tensor_add(dtt[:], x1[:], dtt[:])
                    nc.sync.dma_start(of[t0:t0 + 128, :], dtt[:])
            ctxF.close()
            p_rT.close()
            p_ffnT.close()

    nc.compile()
    return nc


_CACHE = {}


def get_nc(inputs):
    if "nc" not in _CACHE:
        _CACHE["nc"] = build_nc(prep_weights(inputs), ROWS)
    return _CACHE["nc"]


LAST_RESULT = None


def kernel(**inputs):
    global LAST_RESULT
    x = np.asarray(inputs["x"], np.float32)
    nc = get_nc(inputs)
    per = x.shape[0] // NCORES
    in_maps = [{"x": np.ascontiguousarray(x[i * per:(i + 1) * per])}
               for i in range(NCORES)]
    trace = bool(int(os.environ.get("BASSKERNEL_TRACE", "0")))
    res = run_bass_kernel_spmd(nc, in_maps, core_ids=list(range(NCORES)),
                               trace=trace)
    LAST_RESULT = res
    return np.concatenate([np.asarray(r["out"]) for r in res.results], axis=0)

